# revision 3
# baseline (speedup 1.0000x reference)
"""Trainium2 Bass kernel v3 for nn_HadamardTransform: out = value @ (weight + permutation).

v2 + DMA batching (8-tile transfers, 24 HWDGE slots/iter instead of 192) and
vectorized H8 butterflies (strided [128,2,2,2,512] slices: 6 DVE ops per
block instead of 24, perm-adds split DVE/Pool).

    U[i1, j0]    = sum_{i0} (H512[i0,j0]/64) * vT[i1*512+i0, m]   (PE, fp16)
    z[j1*512+j0] = sum_{i1} H8[i1,j1] * U[i1, j0]                 (DVE butterflies)
    o[n]         = z[n] + gP[n],   gP = vT[src, :] pre-permuted on host
"""

import sys

sys.path.insert(0, "/opt/trn_rl_repo")

import numpy as np

import concourse.bacc as bacc
import concourse.bass as bass
import concourse.mybir as mybir
import concourse.tile as tile
from concourse.bass_utils import run_bass_kernel_spmd

ROWS = 8192
N = 4096
N_CORES = 8
MPC = ROWS // N_CORES  # 1024 token rows per core
MC = MPC // 512  # m-chunks of 512
B = 512  # PE Hadamard block (H512)
S = N // B  # 8-way DVE butterfly (H8)
KI = B // 128  # 4 k-subtiles per i1 group
JB = B // 128  # 4 j0 blocks
KT = N // 128  # 32 vT row tiles

# how many of the 8 per-(mc,jb) perm-adds run on Pool (rest on DVE)
POOL_ADDS = 5

_cache = {}

F16 = mybir.dt.float16
F32 = mybir.dt.float32


def _hadamard_pm1(n):
    idx = np.arange(n, dtype=np.int64)
    m = idx[:, None] & idx[None, :]
    pop = np.zeros_like(m)
    for _ in range(int(np.log2(n))):
        pop += m & 1
        m >>= 1
    return np.where(pop % 2 == 0, 1.0, -1.0).astype(np.float32)


def check_structure(weight, permutation):
    """weight must be the scaled Sylvester Hadamard, permutation one-hot."""
    H = _hadamard_pm1(N) / np.sqrt(np.float32(N))
    if not np.array_equal(weight, H):
        return None
    src = np.argmax(permutation, axis=0).astype(np.int32)
    ok = (
        permutation[src, np.arange(N)].min() == 1.0
        and permutation.sum() == N
        and np.abs(permutation).sum() == N
    )
    return src if ok else None


def build_hadamard(reps=1, hw_loop=False, body_reps=1, store_eng='gpsimd', pool_adds=POOL_ADDS):
    """Packed DRAM layouts (host packs/unpacks) so each DMA descriptor is a
    4-8 KB contiguous per-partition run:
      vTp[(mc, i1, p), (ki, m)]  — vt load (mc,i1): [128, KI*512], 4 KB/part
      gPp[(mc, jb, p), (j1, m)]  — gp load (mc,jb): [128, S*512],  8 KB/part
      o  [(mc, jb, p), (j1, m)]  — store  (mc,jb): [128, S*512],  8 KB/part
    """
    nc = bacc.Bacc("TRN2", target_bir_lowering=False)
    vTp = nc.dram_tensor("vTp", (MC * S * 128, KI * 512), F16, kind="ExternalInput")
    gPp = nc.dram_tensor("gPp", (MC * JB * 128, S * 512), F16, kind="ExternalInput")
    hs = nc.dram_tensor("hs", (B, B), F16, kind="ExternalInput")
    o = nc.dram_tensor("o", (MC * JB * 128, S * 512), F16, kind="ExternalOutput")

    add, sub = mybir.AluOpType.add, mybir.AluOpType.subtract

    with tile.TileContext(nc) as tc:
        with (
            tc.tile_pool(name="h", bufs=1) as h_pool,
            tc.tile_pool(name="vt", bufs=2) as vt_pool,
            tc.tile_pool(name="gp", bufs=4) as gp_pool,
            tc.tile_pool(name="ps", bufs=4, space="PSUM") as ps_pool,
            tc.tile_pool(name="u", bufs=2) as u_pool,
            tc.tile_pool(name="s1", bufs=2) as s1_pool,
            tc.tile_pool(name="s2", bufs=2) as s2_pool,
            tc.tile_pool(name="ob", bufs=2) as ob_pool,
        ):
            ht = h_pool.tile([128, KI, JB, 128], F16, tag="h")
            nc.sync.dma_start(
                out=ht,
                in_=hs[:, :].rearrange("(ki p) (jb q) -> p ki jb q", p=128, q=128),
            )

            def issue_loads(mc):
                tiles = []
                for i1 in range(S):
                    vt_t = vt_pool.tile(
                        [128, KI, 512], F16, tag=f"vt{i1}", name=f"vt{i1}"
                    )
                    r0 = (mc * S + i1) * 128
                    nc.sync.dma_start(
                        out=vt_t,
                        in_=vTp[r0 : r0 + 128, :].rearrange(
                            "p (t m) -> p t m", t=KI
                        ),
                    )
                    tiles.append(vt_t)
                return tiles

            def do_chunk(mc, vts):
                m0 = mc * 512
                for jb in range(JB):
                    gp_t = gp_pool.tile([128, S, 512], F16, tag="gp")
                    gr0 = (mc * JB + jb) * 128
                    nc.scalar.dma_start(
                        out=gp_t,
                        in_=gPp[gr0 : gr0 + 128, :].rearrange("p (j m) -> p j m", j=S),
                    )
                    # u[i1] at [:, a, b, c, :] with i1 = 4a + 2b + c
                    u = u_pool.tile([128, 2, 2, 2, 512], F16, tag="u")
                    for i1 in range(S):
                        ps = ps_pool.tile([128, 512], F32, tag="ps")
                        for ki in range(KI):
                            nc.tensor.matmul(
                                out=ps[:, :],
                                lhsT=ht[:, ki, jb, :],
                                rhs=vts[i1][:, ki, :],
                                start=(ki == 0),
                                stop=(ki == KI - 1),
                            )
                        nc.scalar.copy(
                            out=u[:, i1 // 4, (i1 // 2) % 2, i1 % 2, :],
                            in_=ps[:, :],
                        )
                    # H8 butterflies: one strided pair-op per stage half
                    s1 = s1_pool.tile([128, 2, 2, 2, 512], F16, tag="s1")
                    nc.vector.tensor_tensor(
                        out=s1[:, :, :, 0, :], in0=u[:, :, :, 0, :], in1=u[:, :, :, 1, :], op=add
                    )
                    nc.vector.tensor_tensor(
                        out=s1[:, :, :, 1, :], in0=u[:, :, :, 0, :], in1=u[:, :, :, 1, :], op=sub
                    )
                    s2 = s2_pool.tile([128, 2, 2, 2, 512], F16, tag="s2")
                    nc.vector.tensor_tensor(
                        out=s2[:, :, 0, :, :], in0=s1[:, :, 0, :, :], in1=s1[:, :, 1, :, :], op=add
                    )
                    nc.vector.tensor_tensor(
                        out=s2[:, :, 1, :, :], in0=s1[:, :, 0, :, :], in1=s1[:, :, 1, :, :], op=sub
                    )
                    ob = ob_pool.tile([128, 2, 2, 2, 512], F16, tag="ob")
                    nc.vector.tensor_tensor(
                        out=ob[:, 0, :, :, :], in0=s2[:, 0, :, :, :], in1=s2[:, 1, :, :, :], op=add
                    )
                    nc.vector.tensor_tensor(
                        out=ob[:, 1, :, :, :], in0=s2[:, 0, :, :, :], in1=s2[:, 1, :, :, :], op=sub
                    )
                    # perm adds: z[j1] += gP rows; j1 = 4a+2b+c matches ob slice
                    obf = ob[:, :, :, :, :].rearrange("p a b c m -> p (a b c) m")
                    for j1 in range(S):
                        eng = nc.gpsimd if j1 < pool_adds else nc.vector
                        eng.tensor_tensor(
                            out=obf[:, j1, :],
                            in0=obf[:, j1, :],
                            in1=gp_t[:, j1, :],
                            op=add,
                        )
                    getattr(nc, store_eng).dma_start(
                        out=o[(mc * JB + jb) * 128 : (mc * JB + jb + 1) * 128, :],
                        in_=ob[:, :, :, :, :].rearrange("p a b c m -> p (a b c m)"),
                    )

            if hw_loop and reps > 1:
                assert reps % body_reps == 0
                n_chunks = body_reps * MC
                with tc.For_i(0, reps // body_reps) as _i:
                    pending = issue_loads(0)
                    for c in range(n_chunks):
                        cur = pending
                        if c + 1 < n_chunks:
                            pending = issue_loads((c + 1) % MC)
                        do_chunk(c % MC, cur)
            else:
                n_chunks = reps * MC
                pending = issue_loads(0)
                for c in range(n_chunks):
                    cur = pending
                    if c + 1 < n_chunks:
                        pending = issue_loads((c + 1) % MC)
                    do_chunk(c % MC, cur)
    nc.compile()
    return nc


def make_in_maps_h(value, src):
    vT16 = np.ascontiguousarray(value.T.astype(np.float16))  # [N, ROWS]
    gP16 = vT16[src, :]
    hsm = np.ascontiguousarray((_hadamard_pm1(B) / 64.0).astype(np.float16))
    in_maps = []
    for c in range(N_CORES):
        sl = vT16[:, c * MPC : (c + 1) * MPC]
        gl = gP16[:, c * MPC : (c + 1) * MPC]
        vtp = np.ascontiguousarray(
            sl.reshape(S, KI, 128, MC, 512)
            .transpose(3, 0, 2, 1, 4)
            .reshape(MC * S * 128, KI * 512)
        )
        gpp = np.ascontiguousarray(
            gl.reshape(S, JB, 128, MC, 512)
            .transpose(3, 1, 2, 0, 4)
            .reshape(MC * JB * 128, S * 512)
        )
        in_maps.append({"vTp": vtp, "gPp": gpp, "hs": hsm})
    return in_maps


def unpack_out(o_packed):
    """[(mc,jb,p), (j1,m)] packed fp16 -> [MPC, N] fp32 token rows."""
    oT = (
        o_packed.reshape(MC, JB, 128, S, 512)
        .transpose(3, 1, 2, 0, 4)
        .reshape(N, MPC)
    )
    return np.ascontiguousarray(oT.T.astype(np.float32))


# ---------------- dense fallback (arbitrary weight+permutation) ----------------


def build_dense():
    nc = bacc.Bacc("TRN2", target_bir_lowering=False)
    vT = nc.dram_tensor("vT", (N, MPC), mybir.dt.float32r, kind="ExternalInput")
    wgt = nc.dram_tensor("wgt", (N, N), mybir.dt.float32, kind="ExternalInput")
    prm = nc.dram_tensor("prm", (N, N), mybir.dt.float32, kind="ExternalInput")
    o = nc.dram_tensor("o", (N, MPC), mybir.dt.float32, kind="ExternalOutput")
    KTT = N // 128
    NB = N // 128

    with tile.TileContext(nc) as tc:
        with (
            tc.tile_pool(name="vt", bufs=1) as vt_pool,
            tc.tile_pool(name="wp", bufs=2) as wp_pool,
            tc.tile_pool(name="pp", bufs=2) as pp_pool,
            tc.tile_pool(name="ps", bufs=4, space="PSUM") as ps_pool,
            tc.tile_pool(name="os", bufs=4) as os_pool,
        ):
            vts = []
            for t in range(KTT):
                vt_t = vt_pool.tile([128, MPC], mybir.dt.float32r, tag=f"vt{t}")
                nc.sync.dma_start(out=vt_t, in_=vT[t * 128 : (t + 1) * 128, :])
                vts.append(vt_t)

            for nb in range(NB):
                n0 = nb * 128
                wp = wp_pool.tile([128, KTT, 128], mybir.dt.float32r, tag="wp")
                pp = pp_pool.tile([128, KTT, 128], mybir.dt.float32, tag="pp")
                wsrc = wgt[:, n0 : n0 + 128].rearrange("(kt p) j -> p kt j", p=128)
                psrc = prm[:, n0 : n0 + 128].rearrange("(kt p) j -> p kt j", p=128)
                nc.sync.dma_start(out=wp[:, :, :].bitcast(mybir.dt.float32), in_=wsrc)
                nc.sync.dma_start(out=pp, in_=psrc)
                nc.vector.tensor_tensor(
                    out=wp[:, :, :],
                    in0=wp[:, :, :].bitcast(mybir.dt.float32),
                    in1=pp[:, :, :],
                    op=mybir.AluOpType.add,
                )
                for mc in range(MPC // 512):
                    ps = ps_pool.tile([128, 512], mybir.dt.float32, tag="ps")
                    for kt in range(KTT):
                        nc.tensor.matmul(
                            out=ps[:, :],
                            lhsT=wp[:, kt, :],
                            rhs=vts[kt][:, mc * 512 : (mc + 1) * 512],
                            start=(kt == 0),
                            stop=(kt == KTT - 1),
                        )
                    ot = os_pool.tile([128, 512], mybir.dt.float32, tag="os")
                    nc.scalar.copy(out=ot[:, :], in_=ps[:, :])
                    nc.sync.dma_start(
                        out=o[n0 : n0 + 128, mc * 512 : (mc + 1) * 512], in_=ot
                    )
    nc.compile()
    return nc


def make_in_maps(value, weight, permutation):
    vT = np.ascontiguousarray(value.T)
    w = np.ascontiguousarray(weight, dtype=np.float32)
    p = np.ascontiguousarray(permutation, dtype=np.float32)
    in_maps = []
    for c in range(N_CORES):
        in_maps.append(
            {
                "vT": np.ascontiguousarray(vT[:, c * MPC : (c + 1) * MPC]),
                "wgt": w,
                "prm": p,
            }
        )
    return in_maps


def kernel(value, weight, permutation):
    value = np.asarray(value, dtype=np.float32)
    weight = np.asarray(weight, dtype=np.float32)
    permutation = np.asarray(permutation, dtype=np.float32)
    src = check_structure(weight, permutation)
    if src is not None:
        if "had" not in _cache:
            _cache["had"] = build_hadamard()
        nc = _cache["had"]
        in_maps = make_in_maps_h(value, src)
        res = run_bass_kernel_spmd(nc, in_maps, core_ids=list(range(N_CORES)))
        out = np.concatenate(
            [unpack_out(res.results[c]["o"]) for c in range(N_CORES)], axis=0
        )
        return out
    if "dense" not in _cache:
        _cache["dense"] = build_dense()
    nc = _cache["dense"]
    in_maps = make_in_maps(value, weight, permutation)
    res = run_bass_kernel_spmd(nc, in_maps, core_ids=list(range(N_CORES)))
    out = np.concatenate(
        [np.ascontiguousarray(res.results[c]["o"].T) for c in range(N_CORES)], axis=0
    )
    return out
